# revision 1
# baseline (speedup 1.0000x reference)
"""GCNConv kernel: out[i] = sum_{(i,j) in E} vals * (x @ W)[j].

Self-contained fallback implementation. Shapes are fixed by the problem:
x [100000, 256] f32, weight [256, 128] f32, edge_row/edge_col [1600000] i32,
edge_vals [1600000] f32. Output [100000, 128] f32.

Strategy: dense projection via BLAS, then a segment-sum done with a stable
sort by destination row + np.add.reduceat (vectorized, avoids np.add.at's
per-element loop). Edges are processed in chunks to bound the gathered
message buffer.
"""

import numpy as np

N_NODES = 100000
OUT_F = 128


def kernel(x, weight, edge_row, edge_col, edge_vals):
    x = np.ascontiguousarray(x, dtype=np.float32)
    weight = np.ascontiguousarray(weight, dtype=np.float32)
    edge_row = np.asarray(edge_row, dtype=np.int64)
    edge_col = np.asarray(edge_col, dtype=np.int64)
    edge_vals = np.asarray(edge_vals, dtype=np.float32)

    n_nodes = x.shape[0]
    h = x @ weight  # [N, OUT_F]

    try:
        import scipy.sparse as sp

        A = sp.csr_matrix(
            (edge_vals, (edge_row, edge_col)), shape=(n_nodes, n_nodes)
        )
        return np.asarray(A @ h, dtype=np.float32)
    except ImportError:
        pass

    # Sort edges by destination so each segment is contiguous.
    order = np.argsort(edge_row, kind="stable")
    rows_sorted = edge_row[order]
    cols_sorted = edge_col[order]
    vals_sorted = edge_vals[order]

    out = np.zeros((n_nodes, h.shape[1]), dtype=np.float32)

    n_edges = rows_sorted.shape[0]
    chunk = 400000
    start = 0
    while start < n_edges:
        end = min(start + chunk, n_edges)
        # Extend chunk so a destination row never straddles a boundary.
        if end < n_edges:
            last = rows_sorted[end - 1]
            while end < n_edges and rows_sorted[end] == last:
                end += 1
        r = rows_sorted[start:end]
        msg = h[cols_sorted[start:end]]
        msg *= vals_sorted[start:end, None]
        seg_starts = np.concatenate(
            ([0], np.flatnonzero(np.diff(r)) + 1)
        )
        sums = np.add.reduceat(msg, seg_starts, axis=0)
        out[r[seg_starts]] += sums
        start = end

    return out



# revision 2
# speedup vs baseline: 1.1783x; 1.1783x over previous
"""GCNConv on 8 Trainium2 NeuronCores (Bass/Tile).

out[i] = sum_{(i,j) in E} edge_vals * (x @ W)[j]
x [100000, 256] f32, W [256, 128] f32, 1.6M edges -> out [100000, 128] f32.

Sharding: destination rows are partitioned across the 8 cores (12500 each).
Per core:
  phase 1: h = x @ W projected redundantly (bf16, local HBM scratch) on the
    TensorEngine from a host-transposed bf16 copy of x.
  phase 2: the core's edges, grouped by (source col block of 25000 rows,
    dest window of 256 rows) with each group padded to a fixed chunk
    multiple:
      - gpsimd.dma_gather pulls message rows h[col] into SBUF (int16
        block-local indices; calls capped at 1024 descriptors by the SWDGE
        descriptor ring),
      - one batched DVE multiply scales messages by edge values,
      - one batched DVE is_equal builds the one-hot segment matrix
        S[p, s] = (segid[p] == s) per group,
      - TensorEngine accumulates psum[f, s] += msg^T @ S over the group's
        chunks = complete per-window segment sums for one col block,
      - DVE adds psum into a feature-major SBUF accumulator acc[128, 12544].
  out: one plain DMA of acc (feature-major); the host transposes and
  concatenates the 8 shards.

No scatter-add is used anywhere: the hardware's dma_scatter_add loses
updates for duplicate indices issued in one call (concurrent SDMA engines
race on the read-modify-write), so all duplicate-destination accumulation
happens in PSUM/SBUF instead.
"""

import sys

for _p in ("/opt/trn_rl_repo",):
    if _p not in sys.path:
        sys.path.insert(0, _p)

import dataclasses

import numpy as np
import ml_dtypes

import concourse.bacc as bacc
import concourse.mybir as mybir
import concourse.tile as tile

TRACE = False
TRACE_TMPDIR = None
last_exec_time_ns = None


@dataclasses.dataclass
class Cfg:
    n_nodes: int = 100000
    in_f: int = 256
    out_f: int = 128
    n_cores: int = 8
    n_blk: int = 4           # col blocks (block size must fit int16)
    win: int = 256           # dest rows per window (psum free dim)
    sup: int = 2048          # projection supertile (nodes)
    group: int = 0           # slots per (window, block) group; set by prep
    call_slots: int = 1024   # max slots per dma_gather call (SWDGE ring cap)

    @property
    def np_(self):
        return self.n_nodes // self.n_cores

    @property
    def npad(self):
        s = self.sup
        return (self.n_nodes + s - 1) // s * s

    @property
    def blksz(self):
        return -(-self.n_nodes // self.n_blk)

    @property
    def n_win(self):
        return -(-self.np_ // self.win)

    @property
    def npacc(self):
        return self.n_win * self.win

    @property
    def slots(self):
        return self.n_blk * self.n_win * self.group


def host_prep(cfg, x, weight, edge_row, edge_col, edge_vals):
    M, NP, W = cfg.n_cores, cfg.np_, cfg.win
    x = np.asarray(x, np.float32)
    weight = np.asarray(weight, np.float32)
    edge_row = np.asarray(edge_row, np.int64)
    edge_col = np.asarray(edge_col, np.int64)
    edge_vals = np.asarray(edge_vals, np.float32)

    xt = np.zeros((cfg.in_f, cfg.npad), dtype=ml_dtypes.bfloat16)
    xt[:, : cfg.n_nodes] = x.T
    w_bf = weight.astype(ml_dtypes.bfloat16)
    iota = np.ascontiguousarray(
        np.broadcast_to(np.arange(W, dtype=np.float32), (128, W))
    )

    core = edge_row // NP
    blk = edge_col // cfg.blksz
    win = (edge_row % NP) // W
    NWIN = cfg.n_win
    gkey = (core * cfg.n_blk + blk) * NWIN + win
    order = np.lexsort((edge_col, gkey))
    gkey_s = gkey[order]
    col_s = (edge_col[order] % cfg.blksz).astype(np.int16)
    seg_s = ((edge_row[order] % NP) % W).astype(np.float32)
    val_s = edge_vals[order]

    n_groups = M * cfg.n_blk * NWIN
    counts = np.bincount(gkey_s, minlength=n_groups)
    cfg.group = int(-(-counts.max() // 128) * 128)
    E2 = cfg.slots
    starts = np.concatenate(([0], np.cumsum(counts)))

    in_maps = []
    gpc = cfg.n_blk * NWIN
    for c in range(M):
        gidx = np.zeros(E2, np.int16)
        segid = np.zeros(E2, np.float32)
        vals = np.zeros(E2, np.float32)
        for g in range(gpc):
            k = c * gpc + g
            s, e = starts[k], starts[k + 1]
            o = g * cfg.group
            n = e - s
            gidx[o : o + n] = col_s[s:e]
            segid[o : o + n] = seg_s[s:e]
            vals[o : o + n] = val_s[s:e]
        gidx_w = np.tile(gidx.reshape(-1, 16).T, (8, 1))
        segid_w = np.ascontiguousarray(segid.reshape(-1, 128).T)
        vals_w = np.ascontiguousarray(vals.reshape(-1, 128).T).astype(
            ml_dtypes.bfloat16
        )
        in_maps.append(
            {
                "xt": xt,
                "w": w_bf,
                "iota": iota,
                "gidx": np.ascontiguousarray(gidx_w),
                "segid": segid_w,
                "vals": vals_w,
            }
        )
    return in_maps


def build(cfg, debug=False):
    bf16 = mybir.dt.bfloat16
    f32 = mybir.dt.float32
    i16 = mybir.dt.int16
    NP, W, E2 = cfg.np_, cfg.win, cfg.slots
    KB = cfg.in_f // 128
    T = cfg.sup // 128
    GCH = cfg.group // 128
    NWIN = cfg.n_win

    nc = bacc.Bacc("TRN2", target_bir_lowering=False, debug=debug)
    xt_ext = nc.declare_dram_parameter("xt", [cfg.in_f, cfg.npad], bf16, isOutput=False)
    w_ext = nc.declare_dram_parameter("w", [cfg.in_f, cfg.out_f], bf16, isOutput=False)
    iota_ext = nc.declare_dram_parameter("iota", [128, W], f32, isOutput=False)
    gidx_ext = nc.declare_dram_parameter("gidx", [128, E2 // 16], i16, isOutput=False)
    segid_ext = nc.declare_dram_parameter("segid", [128, E2 // 128], f32, isOutput=False)
    vals_ext = nc.declare_dram_parameter("vals", [128, E2 // 128], bf16, isOutput=False)
    out_ext = nc.declare_dram_parameter("out", [128, cfg.npacc], f32, isOutput=True)

    h = nc.dram_tensor("h", [cfg.npad, cfg.out_f], bf16)
    # stage[p, t, :] holds h row s*sup + p*T + t -> 4KB contiguous per
    # partition per supertile (full-rate DMA descriptors)
    hview = h.ap().rearrange("(s p t) e -> s p (t e)", p=128, t=T)

    with tile.TileContext(nc) as tc:
        with (
            tc.tile_pool(name="const", bufs=1) as const_pool,
            tc.tile_pool(name="psum", bufs=4, space="PSUM") as psum_pool,
        ):
            acc = const_pool.tile([128, cfg.npacc], f32)
            nc.vector.memset(acc[:], 0.0)
            w_sb = const_pool.tile([128, KB, cfg.out_f], bf16)
            nc.sync.dma_start(
                out=w_sb[:], in_=w_ext.ap().rearrange("(kb k) f -> k kb f", k=128)
            )
            iota_sb = const_pool.tile([128, W], f32)
            nc.sync.dma_start(out=iota_sb[:], in_=iota_ext[:])
            gidx_sb = const_pool.tile([128, E2 // 16], i16)
            nc.sync.dma_start(out=gidx_sb[:], in_=gidx_ext[:])
            segid_sb = const_pool.tile([128, E2 // 128], f32)
            nc.sync.dma_start(out=segid_sb[:], in_=segid_ext[:])
            vals_sb = const_pool.tile([128, E2 // 128], bf16)
            nc.sync.dma_start(out=vals_sb[:], in_=vals_ext[:])

            # ---- phase 1: projection ----
            with tc.tile_pool(name="proj", bufs=3) as proj_pool:
                xt_t = xt_ext.ap().rearrange("(kb k) n -> kb k n", k=128)
                for s in range(cfg.npad // cfg.sup):
                    xts = []
                    for kb in range(KB):
                        xtile = proj_pool.tile([128, cfg.sup], bf16, tag=f"xt{kb}")
                        nc.sync.dma_start(
                            out=xtile[:],
                            in_=xt_t[kb, :, s * cfg.sup : (s + 1) * cfg.sup],
                        )
                        xts.append(xtile)
                    hstage = proj_pool.tile([128, T, cfg.out_f], bf16, tag="hstage")
                    for t in range(T):
                        ps = psum_pool.tile([128, cfg.out_f], f32, tag="pproj")
                        for kb in range(KB):
                            # psum partition p holds node s*sup + p*T + t
                            nc.tensor.matmul(
                                out=ps[:],
                                lhsT=xts[kb][:, t::T],
                                rhs=w_sb[:, kb, :],
                                start=(kb == 0),
                                stop=(kb == KB - 1),
                            )
                        nc.scalar.copy(hstage[:, t, :], ps[:])
                    nc.sync.dma_start(out=hview[s], in_=hstage[:])

            # ---- phase 2: gather + segment matmul ----
            CS = cfg.call_slots
            with (
                tc.tile_pool(name="msg", bufs=4) as msg_pool,
                tc.tile_pool(name="sval", bufs=4) as sval_pool,
            ):
                for b in range(cfg.n_blk):
                    base_rows = b * cfg.blksz
                    n_rows = min(cfg.blksz, cfg.n_nodes - base_rows)
                    g0 = b * NWIN * cfg.group
                    blk_slots = NWIN * cfg.group
                    msgs = {}
                    n_calls = -(-blk_slots // CS)

                    def issue_call(k):
                        ch = min(CS, blk_slots - k * CS)
                        s0 = g0 + k * CS
                        m = msg_pool.tile(
                            [128, CS // 128, cfg.out_f], bf16, tag="msg"
                        )
                        nc.gpsimd.dma_gather(
                            out_ap=m[:, : ch // 128, :],
                            in_ap=h[base_rows : base_rows + n_rows, :],
                            idxs_ap=gidx_sb[:, s0 // 16 : (s0 + ch) // 16],
                            num_idxs=ch,
                            num_idxs_reg=ch,
                            elem_size=cfg.out_f,
                        )
                        nc.vector.tensor_tensor(
                            out=m[:, : ch // 128, :],
                            in0=m[:, : ch // 128, :],
                            in1=vals_sb[:, s0 // 128 : (s0 + ch) // 128, None]
                            .to_broadcast([128, ch // 128, cfg.out_f]),
                            op=mybir.AluOpType.mult,
                        )
                        msgs[k] = m

                    issued = 0
                    for w_ in range(NWIN):
                        ps = psum_pool.tile([128, W], f32, tag="pseg")
                        sg0 = (g0 + w_ * cfg.group) // 128
                        sval = sval_pool.tile([128, GCH, W], bf16, tag="sval")
                        nc.vector.tensor_tensor(
                            out=sval[:],
                            in0=iota_sb[:, None, :].to_broadcast([128, GCH, W]),
                            in1=segid_sb[:, sg0 : sg0 + GCH, None].to_broadcast(
                                [128, GCH, W]
                            ),
                            op=mybir.AluOpType.is_equal,
                        )
                        for t in range(GCH):
                            ci = w_ * GCH + t
                            k = ci * 128 // CS
                            while issued <= k and issued < n_calls:
                                issue_call(issued)
                                issued += 1
                            ti = ci - k * (CS // 128)
                            nc.tensor.matmul(
                                out=ps[:],
                                lhsT=msgs[k][:, ti, :],
                                rhs=sval[:, t, :],
                                start=(t == 0),
                                stop=(t == GCH - 1),
                            )
                        nc.vector.tensor_add(
                            out=acc[:, w_ * W : (w_ + 1) * W],
                            in0=acc[:, w_ * W : (w_ + 1) * W],
                            in1=ps[:],
                        )

            nc.sync.dma_start(out=out_ext[:], in_=acc[:])
    nc.compile()
    return nc


_cache = {}


def kernel(x, weight, edge_row, edge_col, edge_vals):
    global last_exec_time_ns
    from concourse.bass_utils import run_bass_kernel_spmd

    cfg = Cfg()
    in_maps = host_prep(cfg, x, weight, edge_row, edge_col, edge_vals)
    key = cfg.group
    if key not in _cache:
        _cache[key] = build(cfg, debug=False)
    nc = _cache[key]

    kwargs = {}
    if TRACE:
        kwargs = dict(trace=True, tmpdir=TRACE_TMPDIR)
    res = run_bass_kernel_spmd(
        nc, in_maps, core_ids=list(range(cfg.n_cores)), **kwargs
    )
    last_exec_time_ns = res.exec_time_ns

    outs = []
    for c in range(cfg.n_cores):
        a = np.asarray(res.results[c]["out"], np.float32)
        outs.append(a.T[: cfg.np_, :])
    return np.concatenate(outs, axis=0)


# revision 3
# speedup vs baseline: 1.1881x; 1.0084x over previous
"""GCNConv on 8 Trainium2 NeuronCores (Bass/Tile).

out[i] = sum_{(i,j) in E} edge_vals * (x @ W)[j]
x [100000, 256] f32, W [256, 128] f32, 1.6M edges -> out [100000, 128] f32.

Sharding: destination rows are partitioned across the 8 cores (12500 each).
Per core:
  phase 1: h = x @ W projected redundantly (bf16, local HBM scratch) on the
    TensorEngine from a host-transposed bf16 copy of x.
  phase 2: the core's edges, grouped by (source col block of 25000 rows,
    dest window of 256 rows) with each group padded to a fixed chunk
    multiple:
      - gpsimd.dma_gather pulls message rows h[col] into SBUF (int16
        block-local indices; calls capped at 1024 descriptors by the SWDGE
        descriptor ring),
      - one batched DVE multiply scales messages by edge values,
      - one batched DVE is_equal builds the one-hot segment matrix
        S[p, s] = (segid[p] == s) per group,
      - TensorEngine accumulates psum[f, s] += msg^T @ S over the group's
        chunks = complete per-window segment sums for one col block,
      - DVE adds psum into a feature-major SBUF accumulator acc[128, 12544].
  out: one plain DMA of acc (feature-major); the host transposes and
  concatenates the 8 shards.

No scatter-add is used anywhere: the hardware's dma_scatter_add loses
updates for duplicate indices issued in one call (concurrent SDMA engines
race on the read-modify-write), so all duplicate-destination accumulation
happens in PSUM/SBUF instead.
"""

import sys

for _p in ("/opt/trn_rl_repo",):
    if _p not in sys.path:
        sys.path.insert(0, _p)

import dataclasses

import numpy as np
import ml_dtypes

import concourse.bacc as bacc
import concourse.mybir as mybir
import concourse.tile as tile

TRACE = False
TRACE_TMPDIR = None
last_exec_time_ns = None


@dataclasses.dataclass
class Cfg:
    n_nodes: int = 100000
    in_f: int = 256
    out_f: int = 128
    n_cores: int = 8
    n_blk: int = 4           # col blocks (block size must fit int16)
    win: int = 256           # dest rows per window (psum free dim)
    sup: int = 2048          # projection supertile (nodes)
    group: int = 0           # slots per (window, block) group; set by prep
    call_slots: int = 1024   # max slots per dma_gather call (SWDGE ring cap)

    @property
    def np_(self):
        return self.n_nodes // self.n_cores

    @property
    def blkpad(self):  # padded rows per col block (projection granularity)
        s = self.sup
        return (self.blksz + s - 1) // s * s

    @property
    def npad(self):
        return self.n_blk * self.blkpad

    @property
    def blksz(self):
        return -(-self.n_nodes // self.n_blk)

    @property
    def n_win(self):
        return -(-self.np_ // self.win)

    @property
    def npacc(self):
        return self.n_win * self.win

    @property
    def slots(self):
        return self.n_blk * self.n_win * self.group


def host_prep(cfg, x, weight, edge_row, edge_col, edge_vals):
    M, NP, W = cfg.n_cores, cfg.np_, cfg.win
    x = np.asarray(x, np.float32)
    weight = np.asarray(weight, np.float32)
    edge_row = np.asarray(edge_row, np.int64)
    edge_col = np.asarray(edge_col, np.int64)
    edge_vals = np.asarray(edge_vals, np.float32)

    xt = np.zeros((cfg.in_f, cfg.npad), dtype=ml_dtypes.bfloat16)
    for b in range(cfg.n_blk):
        lo = b * cfg.blksz
        hi = min(lo + cfg.blksz, cfg.n_nodes)
        xt[:, b * cfg.blkpad : b * cfg.blkpad + hi - lo] = x[lo:hi].T
    w_bf = weight.astype(ml_dtypes.bfloat16)
    iota = np.ascontiguousarray(
        np.broadcast_to(np.arange(W, dtype=np.float32), (128, W))
    )

    core = edge_row // NP
    blk = edge_col // cfg.blksz
    win = (edge_row % NP) // W
    NWIN = cfg.n_win
    gkey = (core * cfg.n_blk + blk) * NWIN + win
    order = np.lexsort((edge_col, gkey))
    gkey_s = gkey[order]
    col_s = (edge_col[order] % cfg.blksz).astype(np.int16)
    seg_s = ((edge_row[order] % NP) % W).astype(np.float32)
    val_s = edge_vals[order]

    n_groups = M * cfg.n_blk * NWIN
    counts = np.bincount(gkey_s, minlength=n_groups)
    cfg.group = int(-(-counts.max() // 128) * 128)
    E2 = cfg.slots
    starts = np.concatenate(([0], np.cumsum(counts)))

    in_maps = []
    gpc = cfg.n_blk * NWIN
    for c in range(M):
        gidx = np.zeros(E2, np.int16)
        segid = np.zeros(E2, np.float32)
        vals = np.zeros(E2, np.float32)
        for g in range(gpc):
            k = c * gpc + g
            s, e = starts[k], starts[k + 1]
            o = g * cfg.group
            n = e - s
            gidx[o : o + n] = col_s[s:e]
            segid[o : o + n] = seg_s[s:e]
            vals[o : o + n] = val_s[s:e]
        gidx_w = np.tile(gidx.reshape(-1, 16).T, (8, 1))
        segid_w = np.ascontiguousarray(segid.reshape(-1, 128).T)
        vals_w = np.ascontiguousarray(vals.reshape(-1, 128).T).astype(
            ml_dtypes.bfloat16
        )
        in_maps.append(
            {
                "xt": xt,
                "w": w_bf,
                "iota": iota,
                "gidx": np.ascontiguousarray(gidx_w),
                "segid": segid_w,
                "vals": vals_w,
            }
        )
    return in_maps


def build(cfg, debug=False):
    bf16 = mybir.dt.bfloat16
    f32 = mybir.dt.float32
    i16 = mybir.dt.int16
    NP, W, E2 = cfg.np_, cfg.win, cfg.slots
    KB = cfg.in_f // 128
    T = cfg.sup // 128
    GCH = cfg.group // 128
    NWIN = cfg.n_win

    nc = bacc.Bacc("TRN2", target_bir_lowering=False, debug=debug)
    xt_ext = nc.declare_dram_parameter("xt", [cfg.in_f, cfg.npad], bf16, isOutput=False)
    w_ext = nc.declare_dram_parameter("w", [cfg.in_f, cfg.out_f], bf16, isOutput=False)
    iota_ext = nc.declare_dram_parameter("iota", [128, W], f32, isOutput=False)
    gidx_ext = nc.declare_dram_parameter("gidx", [128, E2 // 16], i16, isOutput=False)
    segid_ext = nc.declare_dram_parameter("segid", [128, E2 // 128], f32, isOutput=False)
    vals_ext = nc.declare_dram_parameter("vals", [128, E2 // 128], bf16, isOutput=False)
    out_ext = nc.declare_dram_parameter("out", [128, cfg.npacc], f32, isOutput=True)

    hs = [
        nc.dram_tensor(f"h{b}", [cfg.blkpad, cfg.out_f], bf16)
        for b in range(cfg.n_blk)
    ]
    # stage[p, t, :] holds h row s*sup + p*T + t -> 4KB contiguous per
    # partition per supertile (full-rate DMA descriptors)
    hviews = [
        hb.ap().rearrange("(s p t) e -> s p (t e)", p=128, t=T) for hb in hs
    ]

    with tile.TileContext(nc) as tc:
        with (
            tc.tile_pool(name="const", bufs=1) as const_pool,
            tc.tile_pool(name="psum", bufs=4, space="PSUM") as psum_pool,
        ):
            acc = const_pool.tile([128, cfg.npacc], f32)
            nc.vector.memset(acc[:], 0.0)
            w_sb = const_pool.tile([128, KB, cfg.out_f], bf16)
            nc.sync.dma_start(
                out=w_sb[:], in_=w_ext.ap().rearrange("(kb k) f -> k kb f", k=128)
            )
            iota_sb = const_pool.tile([128, W], f32)
            nc.sync.dma_start(out=iota_sb[:], in_=iota_ext[:])
            gidx_sb = const_pool.tile([128, E2 // 16], i16)
            nc.sync.dma_start(out=gidx_sb[:], in_=gidx_ext[:])
            segid_sb = const_pool.tile([128, E2 // 128], f32)
            nc.sync.dma_start(out=segid_sb[:], in_=segid_ext[:])
            vals_sb = const_pool.tile([128, E2 // 128], bf16)
            nc.sync.dma_start(out=vals_sb[:], in_=vals_ext[:])

            # ---- phase 1: projection (per col block, so block b's
            # gathers can start while later blocks still project) ----
            with tc.tile_pool(name="proj", bufs=3) as proj_pool:
                xt_t = xt_ext.ap().rearrange("(kb k) n -> kb k n", k=128)
                for b in range(cfg.n_blk):
                    for s in range(cfg.blkpad // cfg.sup):
                        c0 = b * cfg.blkpad + s * cfg.sup
                        xts = []
                        for kb in range(KB):
                            xtile = proj_pool.tile([128, cfg.sup], bf16, tag=f"xt{kb}")
                            nc.sync.dma_start(
                                out=xtile[:],
                                in_=xt_t[kb, :, c0 : c0 + cfg.sup],
                            )
                            xts.append(xtile)
                        hstage = proj_pool.tile([128, T, cfg.out_f], bf16, tag="hstage")
                        for t in range(T):
                            ps = psum_pool.tile([128, cfg.out_f], f32, tag="pproj")
                            for kb in range(KB):
                                # psum partition p holds node s*sup + p*T + t
                                nc.tensor.matmul(
                                    out=ps[:],
                                    lhsT=xts[kb][:, t::T],
                                    rhs=w_sb[:, kb, :],
                                    start=(kb == 0),
                                    stop=(kb == KB - 1),
                                )
                            nc.scalar.copy(hstage[:, t, :], ps[:])
                        nc.sync.dma_start(out=hviews[b][s], in_=hstage[:])

            # ---- phase 2: gather + segment matmul ----
            CS = cfg.call_slots
            with (
                tc.tile_pool(name="msg", bufs=8) as msg_pool,
                tc.tile_pool(name="sval", bufs=6) as sval_pool,
            ):
                for b in range(cfg.n_blk):
                    n_rows = min(cfg.blksz, cfg.n_nodes - b * cfg.blksz)
                    hb = hs[b]
                    g0 = b * NWIN * cfg.group
                    blk_slots = NWIN * cfg.group
                    msgs = {}
                    n_calls = -(-blk_slots // CS)

                    def issue_call(k):
                        ch = min(CS, blk_slots - k * CS)
                        s0 = g0 + k * CS
                        m = msg_pool.tile(
                            [128, CS // 128, cfg.out_f], bf16, tag="msg"
                        )
                        nc.gpsimd.dma_gather(
                            out_ap=m[:, : ch // 128, :],
                            in_ap=hb[:n_rows, :],
                            idxs_ap=gidx_sb[:, s0 // 16 : (s0 + ch) // 16],
                            num_idxs=ch,
                            num_idxs_reg=ch,
                            elem_size=cfg.out_f,
                        )
                        nc.vector.tensor_tensor(
                            out=m[:, : ch // 128, :],
                            in0=m[:, : ch // 128, :],
                            in1=vals_sb[:, s0 // 128 : (s0 + ch) // 128, None]
                            .to_broadcast([128, ch // 128, cfg.out_f]),
                            op=mybir.AluOpType.mult,
                        )
                        msgs[k] = m

                    issued = 0
                    for w_ in range(NWIN):
                        ps = psum_pool.tile([128, W], f32, tag="pseg")
                        sg0 = (g0 + w_ * cfg.group) // 128
                        sval = sval_pool.tile([128, GCH, W], bf16, tag="sval")
                        nc.vector.tensor_tensor(
                            out=sval[:],
                            in0=iota_sb[:, None, :].to_broadcast([128, GCH, W]),
                            in1=segid_sb[:, sg0 : sg0 + GCH, None].to_broadcast(
                                [128, GCH, W]
                            ),
                            op=mybir.AluOpType.is_equal,
                        )
                        for t in range(GCH):
                            ci = w_ * GCH + t
                            k = ci * 128 // CS
                            while issued <= k and issued < n_calls:
                                issue_call(issued)
                                issued += 1
                            ti = ci - k * (CS // 128)
                            nc.tensor.matmul(
                                out=ps[:],
                                lhsT=msgs[k][:, ti, :],
                                rhs=sval[:, t, :],
                                start=(t == 0),
                                stop=(t == GCH - 1),
                            )
                        nc.vector.tensor_add(
                            out=acc[:, w_ * W : (w_ + 1) * W],
                            in0=acc[:, w_ * W : (w_ + 1) * W],
                            in1=ps[:],
                        )

            nc.sync.dma_start(out=out_ext[:], in_=acc[:])
    nc.compile()
    return nc


_cache = {}


def kernel(x, weight, edge_row, edge_col, edge_vals):
    global last_exec_time_ns
    from concourse.bass_utils import run_bass_kernel_spmd

    cfg = Cfg()
    in_maps = host_prep(cfg, x, weight, edge_row, edge_col, edge_vals)
    key = cfg.group
    if key not in _cache:
        _cache[key] = build(cfg, debug=False)
    nc = _cache[key]

    kwargs = {}
    if TRACE:
        kwargs = dict(trace=True, tmpdir=TRACE_TMPDIR)
    res = run_bass_kernel_spmd(
        nc, in_maps, core_ids=list(range(cfg.n_cores)), **kwargs
    )
    last_exec_time_ns = res.exec_time_ns

    outs = []
    for c in range(cfg.n_cores):
        a = np.asarray(res.results[c]["out"], np.float32)
        outs.append(a.T[: cfg.np_, :])
    return np.concatenate(outs, axis=0)


# revision 4
# speedup vs baseline: 1.2655x; 1.0651x over previous
"""GCNConv on 8 Trainium2 NeuronCores (Bass/Tile).

out[i] = sum_{(i,j) in E} edge_vals * (x @ W)[j]
x [100000, 256] f32, W [256, 128] f32, 1.6M edges -> out [100000, 128] f32.

Sharding: destination rows are partitioned across the 8 cores (12500 each).
Per core:
  phase 1: h = x @ W projected redundantly (bf16, local HBM scratch) on the
    TensorEngine from a host-transposed bf16 copy of x.
  phase 2: the core's edges, grouped by (source col block of 25000 rows,
    dest window of 256 rows) with each group padded to a fixed chunk
    multiple:
      - gpsimd.dma_gather pulls message rows h[col] into SBUF (int16
        block-local indices; calls capped at 1024 descriptors by the SWDGE
        descriptor ring),
      - one batched DVE multiply scales messages by edge values,
      - one batched DVE is_equal builds the one-hot segment matrix
        S[p, s] = (segid[p] == s) per group,
      - TensorEngine accumulates psum[f, s] += msg^T @ S over the group's
        chunks = complete per-window segment sums for one col block,
      - DVE adds psum into a feature-major SBUF accumulator acc[128, 12544].
  out: one plain DMA of acc (feature-major); the host transposes and
  concatenates the 8 shards.

No scatter-add is used anywhere: the hardware's dma_scatter_add loses
updates for duplicate indices issued in one call (concurrent SDMA engines
race on the read-modify-write), so all duplicate-destination accumulation
happens in PSUM/SBUF instead.
"""

import sys

for _p in ("/opt/trn_rl_repo",):
    if _p not in sys.path:
        sys.path.insert(0, _p)

import dataclasses

import numpy as np
import ml_dtypes

import concourse.bacc as bacc
import concourse.mybir as mybir
import concourse.tile as tile

TRACE = False
TRACE_TMPDIR = None
last_exec_time_ns = None


@dataclasses.dataclass
class Cfg:
    n_nodes: int = 100000
    in_f: int = 256
    out_f: int = 128
    n_cores: int = 8
    n_blk: int = 4           # col blocks (block size must fit int16)
    win: int = 256           # dest rows per window (psum free dim)
    sup: int = 2048          # projection supertile (nodes)
    group: int = 0           # slots per (window, block) group; set by prep
    call_slots: int = 1024   # max slots per dma_gather call (SWDGE ring cap)

    @property
    def np_(self):
        return self.n_nodes // self.n_cores

    @property
    def blkpad(self):  # padded rows per col block (projection granularity)
        s = self.sup
        return (self.blksz + s - 1) // s * s

    @property
    def npad(self):
        return self.n_blk * self.blkpad

    @property
    def blksz(self):
        return -(-self.n_nodes // self.n_blk)

    @property
    def n_win(self):
        return -(-self.np_ // self.win)

    @property
    def npacc(self):
        return self.n_win * self.win

    @property
    def slots(self):
        return self.n_blk * self.n_win * self.group


def host_prep(cfg, x, weight, edge_row, edge_col, edge_vals):
    M, NP, W = cfg.n_cores, cfg.np_, cfg.win
    x = np.asarray(x, np.float32)
    weight = np.asarray(weight, np.float32)
    edge_row = np.asarray(edge_row, np.int64)
    edge_col = np.asarray(edge_col, np.int64)
    edge_vals = np.asarray(edge_vals, np.float32)

    xt = np.zeros((cfg.in_f, cfg.npad), dtype=ml_dtypes.bfloat16)
    for b in range(cfg.n_blk):
        lo = b * cfg.blksz
        hi = min(lo + cfg.blksz, cfg.n_nodes)
        xt[:, b * cfg.blkpad : b * cfg.blkpad + hi - lo] = x[lo:hi].T
    w_bf = weight.astype(ml_dtypes.bfloat16)
    iota = np.ascontiguousarray(
        np.broadcast_to(np.arange(W, dtype=np.float32), (128, W))
    )

    core = edge_row // NP
    blk = edge_col // cfg.blksz
    win = (edge_row % NP) // W
    NWIN = cfg.n_win
    gkey = (core * cfg.n_blk + blk) * NWIN + win
    order = np.lexsort((edge_col, gkey))
    gkey_s = gkey[order]
    col_s = (edge_col[order] % cfg.blksz).astype(np.int16)
    seg_s = ((edge_row[order] % NP) % W).astype(np.float32)
    val_s = edge_vals[order]

    n_groups = M * cfg.n_blk * NWIN
    counts = np.bincount(gkey_s, minlength=n_groups)
    cfg.group = int(-(-counts.max() // 128) * 128)
    E2 = cfg.slots
    starts = np.concatenate(([0], np.cumsum(counts)))

    in_maps = []
    gpc = cfg.n_blk * NWIN
    for c in range(M):
        gidx = np.zeros(E2, np.int16)
        segid = np.zeros(E2, np.float32)
        vals = np.zeros(E2, np.float32)
        for g in range(gpc):
            k = c * gpc + g
            s, e = starts[k], starts[k + 1]
            o = g * cfg.group
            n = e - s
            gidx[o : o + n] = col_s[s:e]
            segid[o : o + n] = seg_s[s:e]
            vals[o : o + n] = val_s[s:e]
        gidx_w = np.tile(gidx.reshape(-1, 16).T, (8, 1))
        segid_w = np.ascontiguousarray(segid.reshape(-1, 128).T)
        vals_w = np.ascontiguousarray(vals.reshape(-1, 128).T).astype(
            ml_dtypes.bfloat16
        )
        in_maps.append(
            {
                "xt": xt,
                "w": w_bf,
                "iota": iota,
                "gidx": np.ascontiguousarray(gidx_w),
                "segid": segid_w,
                "vals": vals_w,
            }
        )
    return in_maps


def build(cfg, debug=False):
    bf16 = mybir.dt.bfloat16
    f32 = mybir.dt.float32
    i16 = mybir.dt.int16
    NP, W, E2 = cfg.np_, cfg.win, cfg.slots
    KB = cfg.in_f // 128
    T = cfg.sup // 128
    GCH = cfg.group // 128
    NWIN = cfg.n_win

    nc = bacc.Bacc("TRN2", target_bir_lowering=False, debug=debug)
    xt_ext = nc.declare_dram_parameter("xt", [cfg.in_f, cfg.npad], bf16, isOutput=False)
    w_ext = nc.declare_dram_parameter("w", [cfg.in_f, cfg.out_f], bf16, isOutput=False)
    iota_ext = nc.declare_dram_parameter("iota", [128, W], f32, isOutput=False)
    gidx_ext = nc.declare_dram_parameter("gidx", [128, E2 // 16], i16, isOutput=False)
    segid_ext = nc.declare_dram_parameter("segid", [128, E2 // 128], f32, isOutput=False)
    vals_ext = nc.declare_dram_parameter("vals", [128, E2 // 128], bf16, isOutput=False)
    out_ext = nc.declare_dram_parameter("out", [128, cfg.npacc], f32, isOutput=True)

    hs = [
        nc.dram_tensor(f"h{b}", [cfg.blkpad, cfg.out_f], bf16)
        for b in range(cfg.n_blk)
    ]
    # stage[p, t, :] holds h row s*sup + p*T + t -> 4KB contiguous per
    # partition per supertile (full-rate DMA descriptors)
    hviews = [
        hb.ap().rearrange("(s p t) e -> s p (t e)", p=128, t=T) for hb in hs
    ]

    with tile.TileContext(nc) as tc:
        with (
            tc.tile_pool(name="const", bufs=1) as const_pool,
            tc.tile_pool(name="psum", bufs=4, space="PSUM") as psum_pool,
        ):
            acc = const_pool.tile([128, cfg.npacc], f32)
            nc.vector.memset(acc[:], 0.0)
            w_sb = const_pool.tile([128, KB, cfg.out_f], bf16)
            nc.sync.dma_start(
                out=w_sb[:], in_=w_ext.ap().rearrange("(kb k) f -> k kb f", k=128)
            )
            iota_sb = const_pool.tile([128, W], f32)
            nc.sync.dma_start(out=iota_sb[:], in_=iota_ext[:])
            gidx_sb = const_pool.tile([128, E2 // 16], i16)
            nc.sync.dma_start(out=gidx_sb[:], in_=gidx_ext[:])
            segid_sb = const_pool.tile([128, E2 // 128], f32)
            nc.sync.dma_start(out=segid_sb[:], in_=segid_ext[:])
            vals_sb = const_pool.tile([128, E2 // 128], bf16)
            nc.sync.dma_start(out=vals_sb[:], in_=vals_ext[:])

            # ---- phase 1: projection (per col block, so block b's
            # gathers can start while later blocks still project).
            # NOTE: proj/msg/sval pools are all open at once so phase-2
            # tiles do NOT reuse phase-1 SBUF addresses -- reuse would add
            # WAR edges serializing the first gather behind the whole
            # projection (measured: 258us startup stall).
            with (
                tc.tile_pool(name="proj", bufs=3) as proj_pool,
                tc.tile_pool(name="msg", bufs=8) as msg_pool,
                tc.tile_pool(name="sval", bufs=6) as sval_pool,
            ):
                xt_t = xt_ext.ap().rearrange("(kb k) n -> kb k n", k=128)
                for b in range(cfg.n_blk):
                    for s in range(cfg.blkpad // cfg.sup):
                        c0 = b * cfg.blkpad + s * cfg.sup
                        xts = []
                        for kb in range(KB):
                            xtile = proj_pool.tile([128, cfg.sup], bf16, tag=f"xt{kb}")
                            nc.sync.dma_start(
                                out=xtile[:],
                                in_=xt_t[kb, :, c0 : c0 + cfg.sup],
                            )
                            xts.append(xtile)
                        hstage = proj_pool.tile([128, T, cfg.out_f], bf16, tag="hstage")
                        for t in range(T):
                            ps = psum_pool.tile([128, cfg.out_f], f32, tag="pproj")
                            for kb in range(KB):
                                # psum partition p holds node s*sup + p*T + t
                                nc.tensor.matmul(
                                    out=ps[:],
                                    lhsT=xts[kb][:, t::T],
                                    rhs=w_sb[:, kb, :],
                                    start=(kb == 0),
                                    stop=(kb == KB - 1),
                                )
                            nc.scalar.copy(hstage[:, t, :], ps[:])
                        nc.sync.dma_start(out=hviews[b][s], in_=hstage[:])

                # ---- phase 2: gather + segment matmul ----
                CS = cfg.call_slots
                for b in range(cfg.n_blk):
                    n_rows = min(cfg.blksz, cfg.n_nodes - b * cfg.blksz)
                    hb = hs[b]
                    g0 = b * NWIN * cfg.group
                    blk_slots = NWIN * cfg.group
                    msgs = {}
                    n_calls = -(-blk_slots // CS)

                    def issue_call(k):
                        ch = min(CS, blk_slots - k * CS)
                        s0 = g0 + k * CS
                        m = msg_pool.tile(
                            [128, CS // 128, cfg.out_f], bf16, tag="msg"
                        )
                        nc.gpsimd.dma_gather(
                            out_ap=m[:, : ch // 128, :],
                            in_ap=hb[:n_rows, :],
                            idxs_ap=gidx_sb[:, s0 // 16 : (s0 + ch) // 16],
                            num_idxs=ch,
                            num_idxs_reg=ch,
                            elem_size=cfg.out_f,
                        )
                        nc.vector.tensor_tensor(
                            out=m[:, : ch // 128, :],
                            in0=m[:, : ch // 128, :],
                            in1=vals_sb[:, s0 // 128 : (s0 + ch) // 128, None]
                            .to_broadcast([128, ch // 128, cfg.out_f]),
                            op=mybir.AluOpType.mult,
                        )
                        msgs[k] = m

                    issued = 0
                    for w_ in range(NWIN):
                        ps = psum_pool.tile([128, W], f32, tag="pseg")
                        sg0 = (g0 + w_ * cfg.group) // 128
                        sval = sval_pool.tile([128, GCH, W], bf16, tag="sval")
                        nc.vector.tensor_tensor(
                            out=sval[:],
                            in0=iota_sb[:, None, :].to_broadcast([128, GCH, W]),
                            in1=segid_sb[:, sg0 : sg0 + GCH, None].to_broadcast(
                                [128, GCH, W]
                            ),
                            op=mybir.AluOpType.is_equal,
                        )
                        for t in range(GCH):
                            ci = w_ * GCH + t
                            k = ci * 128 // CS
                            while issued <= k and issued < n_calls:
                                issue_call(issued)
                                issued += 1
                            ti = ci - k * (CS // 128)
                            nc.tensor.matmul(
                                out=ps[:],
                                lhsT=msgs[k][:, ti, :],
                                rhs=sval[:, t, :],
                                start=(t == 0),
                                stop=(t == GCH - 1),
                            )
                        nc.vector.tensor_add(
                            out=acc[:, w_ * W : (w_ + 1) * W],
                            in0=acc[:, w_ * W : (w_ + 1) * W],
                            in1=ps[:],
                        )

            nc.sync.dma_start(out=out_ext[:], in_=acc[:])
    nc.compile()
    return nc


_cache = {}


def kernel(x, weight, edge_row, edge_col, edge_vals):
    global last_exec_time_ns
    from concourse.bass_utils import run_bass_kernel_spmd

    cfg = Cfg()
    in_maps = host_prep(cfg, x, weight, edge_row, edge_col, edge_vals)
    key = cfg.group
    if key not in _cache:
        _cache[key] = build(cfg, debug=False)
    nc = _cache[key]

    kwargs = {}
    if TRACE:
        kwargs = dict(trace=True, tmpdir=TRACE_TMPDIR)
    res = run_bass_kernel_spmd(
        nc, in_maps, core_ids=list(range(cfg.n_cores)), **kwargs
    )
    last_exec_time_ns = res.exec_time_ns

    outs = []
    for c in range(cfg.n_cores):
        a = np.asarray(res.results[c]["out"], np.float32)
        outs.append(a.T[: cfg.np_, :])
    return np.concatenate(outs, axis=0)


# revision 5
# speedup vs baseline: 1.2827x; 1.0136x over previous
"""GCNConv on 8 Trainium2 NeuronCores (Bass/Tile).

out[i] = sum_{(i,j) in E} edge_vals * (x @ W)[j]
x [100000, 256] f32, W [256, 128] f32, 1.6M edges -> out [100000, 128] f32.

Sharding: destination rows are partitioned across the 8 cores (12500 each).
Per core:
  phase 1: h = x @ W projected redundantly (bf16, local HBM scratch) on the
    TensorEngine from a host-transposed bf16 copy of x.
  phase 2: the core's edges, grouped by (source col block of 25000 rows,
    dest window of 256 rows) with each group padded to a fixed chunk
    multiple:
      - gpsimd.dma_gather pulls message rows h[col] into SBUF (int16
        block-local indices; calls capped at 1024 descriptors by the SWDGE
        descriptor ring),
      - one batched DVE multiply scales messages by edge values,
      - one batched DVE is_equal builds the one-hot segment matrix
        S[p, s] = (segid[p] == s) per group,
      - TensorEngine accumulates psum[f, s] += msg^T @ S over the group's
        chunks = complete per-window segment sums for one col block,
      - DVE adds psum into a feature-major SBUF accumulator acc[128, 12544].
  out: one plain DMA of acc (feature-major); the host transposes and
  concatenates the 8 shards.

No scatter-add is used anywhere: the hardware's dma_scatter_add loses
updates for duplicate indices issued in one call (concurrent SDMA engines
race on the read-modify-write), so all duplicate-destination accumulation
happens in PSUM/SBUF instead.
"""

import sys

for _p in ("/opt/trn_rl_repo",):
    if _p not in sys.path:
        sys.path.insert(0, _p)

import dataclasses

import numpy as np
import ml_dtypes

import concourse.bacc as bacc
import concourse.mybir as mybir
import concourse.tile as tile

TRACE = False
TRACE_TMPDIR = None
last_exec_time_ns = None


@dataclasses.dataclass
class Cfg:
    n_nodes: int = 100000
    in_f: int = 256
    out_f: int = 128
    n_cores: int = 8
    n_blk: int = 4           # col blocks (block size must fit int16)
    win: int = 256           # dest rows per window (psum free dim)
    sup: int = 2048          # projection supertile (nodes)
    group: int = 0           # slots per (window, block) group; set by prep
    call_slots: int = 1024   # max slots per dma_gather call (SWDGE ring cap)

    @property
    def np_(self):
        return self.n_nodes // self.n_cores

    @property
    def blkpad(self):  # padded rows per col block (projection granularity)
        s = self.sup
        return (self.blksz + s - 1) // s * s

    @property
    def npad(self):
        return self.n_blk * self.blkpad

    @property
    def blksz(self):
        return -(-self.n_nodes // self.n_blk)

    @property
    def n_win(self):
        return -(-self.np_ // self.win)

    @property
    def npacc(self):
        return self.n_win * self.win

    @property
    def slots(self):
        return self.n_blk * self.n_win * self.group


def host_prep(cfg, x, weight, edge_row, edge_col, edge_vals):
    M, NP, W = cfg.n_cores, cfg.np_, cfg.win
    x = np.asarray(x, np.float32)
    weight = np.asarray(weight, np.float32)
    edge_row = np.asarray(edge_row, np.int64)
    edge_col = np.asarray(edge_col, np.int64)
    edge_vals = np.asarray(edge_vals, np.float32)

    xt = np.zeros((cfg.in_f, cfg.npad), dtype=ml_dtypes.bfloat16)
    for b in range(cfg.n_blk):
        lo = b * cfg.blksz
        hi = min(lo + cfg.blksz, cfg.n_nodes)
        xt[:, b * cfg.blkpad : b * cfg.blkpad + hi - lo] = x[lo:hi].T
    w_bf = weight.astype(ml_dtypes.bfloat16)
    iota = np.ascontiguousarray(
        np.broadcast_to(np.arange(W, dtype=np.float32), (128, W))
    )

    core = edge_row // NP
    blk = edge_col // cfg.blksz
    win = (edge_row % NP) // W
    NWIN = cfg.n_win
    gkey = (core * cfg.n_blk + blk) * NWIN + win
    order = np.lexsort((edge_col, gkey))
    gkey_s = gkey[order]
    col_s = (edge_col[order] % cfg.blksz).astype(np.int16)
    seg_s = ((edge_row[order] % NP) % W).astype(np.float32)
    val_s = edge_vals[order]

    n_groups = M * cfg.n_blk * NWIN
    counts = np.bincount(gkey_s, minlength=n_groups)
    cfg.group = int(-(-counts.max() // 128) * 128)
    E2 = cfg.slots
    starts = np.concatenate(([0], np.cumsum(counts)))

    in_maps = []
    gpc = cfg.n_blk * NWIN
    for c in range(M):
        gidx = np.zeros(E2, np.int16)
        segid = np.zeros(E2, np.float32)
        vals = np.zeros(E2, np.float32)
        for g in range(gpc):
            k = c * gpc + g
            s, e = starts[k], starts[k + 1]
            o = g * cfg.group
            n = e - s
            gidx[o : o + n] = col_s[s:e]
            segid[o : o + n] = seg_s[s:e]
            vals[o : o + n] = val_s[s:e]
        gidx_w = np.tile(gidx.reshape(-1, 16).T, (8, 1))
        segid_w = np.ascontiguousarray(segid.reshape(-1, 128).T)
        vals_w = np.ascontiguousarray(vals.reshape(-1, 128).T).astype(
            ml_dtypes.bfloat16
        )
        in_maps.append(
            {
                "xt": xt,
                "w": w_bf,
                "iota": iota,
                "gidx": np.ascontiguousarray(gidx_w),
                "segid": segid_w,
                "vals": vals_w,
            }
        )
    return in_maps


def build(cfg, debug=False):
    bf16 = mybir.dt.bfloat16
    f32 = mybir.dt.float32
    i16 = mybir.dt.int16
    NP, W, E2 = cfg.np_, cfg.win, cfg.slots
    KB = cfg.in_f // 128
    T = cfg.sup // 128
    GCH = cfg.group // 128
    NWIN = cfg.n_win

    nc = bacc.Bacc("TRN2", target_bir_lowering=False, debug=debug)
    xt_ext = nc.declare_dram_parameter("xt", [cfg.in_f, cfg.npad], bf16, isOutput=False)
    w_ext = nc.declare_dram_parameter("w", [cfg.in_f, cfg.out_f], bf16, isOutput=False)
    iota_ext = nc.declare_dram_parameter("iota", [128, W], f32, isOutput=False)
    gidx_ext = nc.declare_dram_parameter("gidx", [128, E2 // 16], i16, isOutput=False)
    segid_ext = nc.declare_dram_parameter("segid", [128, E2 // 128], f32, isOutput=False)
    vals_ext = nc.declare_dram_parameter("vals", [128, E2 // 128], bf16, isOutput=False)
    out_ext = nc.declare_dram_parameter("out", [128, cfg.npacc], f32, isOutput=True)

    hs = [
        nc.dram_tensor(f"h{b}", [cfg.blkpad, cfg.out_f], bf16)
        for b in range(cfg.n_blk)
    ]
    # stage[p, t, :] holds h row s*sup + p*T + t -> 4KB contiguous per
    # partition per supertile (full-rate DMA descriptors)
    hviews = [
        hb.ap().rearrange("(s p t) e -> s p (t e)", p=128, t=T) for hb in hs
    ]

    with tile.TileContext(nc) as tc:
        with (
            tc.tile_pool(name="const", bufs=1) as const_pool,
            tc.tile_pool(name="psum", bufs=4, space="PSUM") as psum_pool,
        ):
            acc = const_pool.tile([128, cfg.npacc], f32)
            nc.vector.memset(acc[:], 0.0)
            # dummy 16-idx gather: forces the Q7 extended-inst library
            # reload at t~0 instead of on the critical first real gather
            dummy_idx = const_pool.tile([128, 1], i16)
            nc.gpsimd.memset(dummy_idx[:], 0)
            scrap = const_pool.tile([128, 1, cfg.out_f], bf16)
            nc.gpsimd.dma_gather(
                out_ap=scrap[:],
                in_ap=xt_ext.ap().rearrange("f (r c) -> (f r) c", c=cfg.out_f),
                idxs_ap=dummy_idx[:],
                num_idxs=16,
                num_idxs_reg=16,
                elem_size=cfg.out_f,
            )
            w_sb = const_pool.tile([128, KB, cfg.out_f], bf16)
            nc.sync.dma_start(
                out=w_sb[:], in_=w_ext.ap().rearrange("(kb k) f -> k kb f", k=128)
            )
            iota_sb = const_pool.tile([128, W], f32)
            nc.sync.dma_start(out=iota_sb[:], in_=iota_ext[:])
            gidx_sb = const_pool.tile([128, E2 // 16], i16)
            nc.sync.dma_start(out=gidx_sb[:], in_=gidx_ext[:])
            segid_sb = const_pool.tile([128, E2 // 128], f32)
            nc.sync.dma_start(out=segid_sb[:], in_=segid_ext[:])
            vals_sb = const_pool.tile([128, E2 // 128], bf16)
            nc.sync.dma_start(out=vals_sb[:], in_=vals_ext[:])

            # ---- phase 1: projection (per col block, so block b's
            # gathers can start while later blocks still project).
            # NOTE: proj/msg/sval pools are all open at once so phase-2
            # tiles do NOT reuse phase-1 SBUF addresses -- reuse would add
            # WAR edges serializing the first gather behind the whole
            # projection (measured: 258us startup stall).
            with (
                tc.tile_pool(name="proj", bufs=3) as proj_pool,
                tc.tile_pool(name="msg", bufs=8) as msg_pool,
                tc.tile_pool(name="sval", bufs=6) as sval_pool,
            ):
                xt_t = xt_ext.ap().rearrange("(kb k) n -> kb k n", k=128)
                for b in range(cfg.n_blk):
                    for s in range(cfg.blkpad // cfg.sup):
                        c0 = b * cfg.blkpad + s * cfg.sup
                        xts = []
                        for kb in range(KB):
                            xtile = proj_pool.tile([128, cfg.sup], bf16, tag=f"xt{kb}")
                            nc.sync.dma_start(
                                out=xtile[:],
                                in_=xt_t[kb, :, c0 : c0 + cfg.sup],
                            )
                            xts.append(xtile)
                        hstage = proj_pool.tile([128, T, cfg.out_f], bf16, tag="hstage")
                        for t in range(T):
                            ps = psum_pool.tile([128, cfg.out_f], f32, tag="pproj")
                            for kb in range(KB):
                                # psum partition p holds node s*sup + p*T + t
                                nc.tensor.matmul(
                                    out=ps[:],
                                    lhsT=xts[kb][:, t::T],
                                    rhs=w_sb[:, kb, :],
                                    start=(kb == 0),
                                    stop=(kb == KB - 1),
                                )
                            if t % 2 == 0:
                                nc.scalar.copy(hstage[:, t, :], ps[:])
                            else:
                                nc.vector.tensor_copy(hstage[:, t, :], ps[:])
                        nc.sync.dma_start(out=hviews[b][s], in_=hstage[:])

                # ---- phase 2: gather + segment matmul ----
                CS = cfg.call_slots
                for b in range(cfg.n_blk):
                    n_rows = min(cfg.blksz, cfg.n_nodes - b * cfg.blksz)
                    hb = hs[b]
                    g0 = b * NWIN * cfg.group
                    blk_slots = NWIN * cfg.group
                    msgs = {}
                    n_calls = -(-blk_slots // CS)

                    def issue_call(k):
                        ch = min(CS, blk_slots - k * CS)
                        s0 = g0 + k * CS
                        m = msg_pool.tile(
                            [128, CS // 128, cfg.out_f], bf16, tag="msg"
                        )
                        nc.gpsimd.dma_gather(
                            out_ap=m[:, : ch // 128, :],
                            in_ap=hb[:n_rows, :],
                            idxs_ap=gidx_sb[:, s0 // 16 : (s0 + ch) // 16],
                            num_idxs=ch,
                            num_idxs_reg=ch,
                            elem_size=cfg.out_f,
                        )
                        nc.vector.tensor_tensor(
                            out=m[:, : ch // 128, :],
                            in0=m[:, : ch // 128, :],
                            in1=vals_sb[:, s0 // 128 : (s0 + ch) // 128, None]
                            .to_broadcast([128, ch // 128, cfg.out_f]),
                            op=mybir.AluOpType.mult,
                        )
                        msgs[k] = m

                    issued = 0
                    for w_ in range(NWIN):
                        ps = psum_pool.tile([128, W], f32, tag="pseg")
                        sg0 = (g0 + w_ * cfg.group) // 128
                        sval = sval_pool.tile([128, GCH, W], bf16, tag="sval")
                        nc.vector.tensor_tensor(
                            out=sval[:],
                            in0=iota_sb[:, None, :].to_broadcast([128, GCH, W]),
                            in1=segid_sb[:, sg0 : sg0 + GCH, None].to_broadcast(
                                [128, GCH, W]
                            ),
                            op=mybir.AluOpType.is_equal,
                        )
                        for t in range(GCH):
                            ci = w_ * GCH + t
                            k = ci * 128 // CS
                            while issued <= k and issued < n_calls:
                                issue_call(issued)
                                issued += 1
                            ti = ci - k * (CS // 128)
                            nc.tensor.matmul(
                                out=ps[:],
                                lhsT=msgs[k][:, ti, :],
                                rhs=sval[:, t, :],
                                start=(t == 0),
                                stop=(t == GCH - 1),
                            )
                        nc.vector.tensor_add(
                            out=acc[:, w_ * W : (w_ + 1) * W],
                            in0=acc[:, w_ * W : (w_ + 1) * W],
                            in1=ps[:],
                        )
                        if b == cfg.n_blk - 1:
                            # window w is now complete: stream it out
                            nc.sync.dma_start(
                                out=out_ext[:, w_ * W : (w_ + 1) * W],
                                in_=acc[:, w_ * W : (w_ + 1) * W],
                            )
    nc.compile()
    return nc


_cache = {}


def kernel(x, weight, edge_row, edge_col, edge_vals):
    global last_exec_time_ns
    from concourse.bass_utils import run_bass_kernel_spmd

    cfg = Cfg()
    in_maps = host_prep(cfg, x, weight, edge_row, edge_col, edge_vals)
    key = cfg.group
    if key not in _cache:
        _cache[key] = build(cfg, debug=False)
    nc = _cache[key]

    kwargs = {}
    if TRACE:
        kwargs = dict(trace=True, tmpdir=TRACE_TMPDIR)
    res = run_bass_kernel_spmd(
        nc, in_maps, core_ids=list(range(cfg.n_cores)), **kwargs
    )
    last_exec_time_ns = res.exec_time_ns

    outs = []
    for c in range(cfg.n_cores):
        a = np.asarray(res.results[c]["out"], np.float32)
        outs.append(a.T[: cfg.np_, :])
    return np.concatenate(outs, axis=0)
